# revision 1
# baseline (speedup 1.0000x reference)
"""Trainium2 Bass kernel for nn_DLI_loss_full.

Key algebraic fact: logits[b,j,k] = hw[b,j] + xw[b,k] and the loss is
sum(lse - tgt) over valid groups, so the hw[b,j] term (the whole LSTM
path) cancels exactly:

    per_group[b,j] = log(sum_{k=j+1}^{len_b-1} exp(xw[b,k])) - xw[b,j+1]
    loss = sum(per_group) / sum_b(len_b - 1)

with xw = encoder_output @ w_fc[HID:].  The kernel therefore only
streams encoder_output once (memory-bound), computes xw via
multiply+256-wide reductions, then gets every suffix log-sum-exp with
one hardware suffix-sum scan per 48-wide chunk plus a cross-chunk
combine done as a tiny 128x128 matmul.

Per-core layout: 16 batches x 8 chunks of 48 timesteps = 128 SBUF
partitions, each partition's encoder slice contiguous in DRAM.  All
encoder DMAs ride one HWDGE queue (a single queue sustains ~340 B/ns;
splitting queues loses aggregate bandwidth).
"""

from contextlib import ExitStack

import numpy as np

import concourse.bacc as bacc
import concourse.mybir as mybir
import concourse.tile as tile
from concourse import bass_utils

B, T, D, HID = 128, 384, 256, 256
NCORES = 8
BS = B // NCORES            # 16 batches per core
CH = 8                      # chunks per sequence
L = T // CH                 # 48 timesteps per chunk
P = BS * CH                 # 128 partitions
NP = 8                      # DMA/compute pieces along the free axis
LP = L // NP                # 6 timesteps per piece
F32 = mybir.dt.float32
I32 = mybir.dt.int32
EPS = 1e-30                 # keeps ln() finite on fully-masked tails

# pieces whose multiply runs on gpsimd (plain tensor_tensor + vector
# reduce); late pieces multiply on the faster vector engine so the
# post-DMA tail stays short.
MUL_ON_GPSIMD = (0, 1, 2, 3, 4)

_cache = {}


def _build_nc():
    nc = bacc.Bacc(
        "TRN2", target_bir_lowering=False, debug=False, num_devices=NCORES
    )
    x = nc.dram_tensor("x", [BS, T, D], F32, kind="ExternalInput").ap()
    mk = nc.dram_tensor("mk", [BS, T], I32, kind="ExternalInput").ap()
    wt = nc.dram_tensor("wt", [P, D], F32, kind="ExternalInput").ap()
    um = nc.dram_tensor("um", [P, P], F32, kind="ExternalInput").ap()
    cm = nc.dram_tensor("cm", [P, L], F32, kind="ExternalInput").ap()
    out = nc.dram_tensor("out", [P, 2], F32, kind="ExternalOutput").ap()

    add = mybir.AluOpType.add
    mult = mybir.AluOpType.mult
    bypass = mybir.AluOpType.bypass
    AX = mybir.AxisListType.X
    ACT = mybir.ActivationFunctionType

    with tile.TileContext(nc) as tc, ExitStack() as ctx:
        sp = ctx.enter_context(tc.tile_pool(name="small", bufs=1))
        xp = ctx.enter_context(tc.tile_pool(name="xp", bufs=NP))
        pp = ctx.enter_context(tc.tile_pool(name="psum", bufs=2, space="PSUM"))

        # x-piece loads first, all on the scalar HWDGE queue: it has
        # arbitration priority and sustains ~287 B/ns even while engines
        # read the landing tiles (the sync queue collapses to ~80-185 in
        # the same conditions)
        x_p = x.rearrange("b (c n l) d -> (b c) n (l d)", c=CH, n=NP)
        xts = []
        for i in range(NP):
            xt = xp.tile([P, LP * D], F32, tag="x")
            nc.scalar.dma_start(xt[:], x_p[:, i, :])
            xts.append(xt)

        # small constants ride the gpsimd SWDGE queue so they don't take
        # issue slots from the big stream
        w_sb = sp.tile([P, D], F32)
        nc.gpsimd.dma_start(w_sb[:], wt)
        cm_sb = sp.tile([P, L], F32)
        nc.gpsimd.dma_start(cm_sb[:], cm)
        mi = sp.tile([P, L], I32)
        nc.gpsimd.dma_start(mi[:], mk.rearrange("b (c l) -> (b c) l", c=CH))
        u_sb = sp.tile([P, P], F32)
        nc.gpsimd.dma_start(u_sb[:], um)
        mf = sp.tile([P, L], F32)
        nc.gpsimd.tensor_copy(mf[:], mi[:])

        # warm the Exp activation table while DMA streams
        warm = sp.tile([P, 1], F32)
        nc.scalar.activation(warm[:], cm_sb[:, 0:1], ACT.Exp)

        # replicate w LP times on-chip so the per-piece multiply reads a
        # plain contiguous operand (0-stride broadcast halves DVE rate,
        # and gpsimd cannot read PSUM)
        wrep = sp.tile([P, LP * D], F32)
        nc.vector.tensor_copy(wrep[:, 0:D], w_sb[:])
        nc.vector.tensor_copy(wrep[:, D:2 * D], wrep[:, 0:D])
        nc.vector.tensor_copy(wrep[:, 2 * D:4 * D], wrep[:, 0:2 * D])
        nc.vector.tensor_copy(wrep[:, 4 * D:6 * D], wrep[:, 2 * D:4 * D])
        w3 = wrep[:].rearrange("p (l d) -> p l d", d=D)

        # xw[p, t] = sum_d x[p, t, d] * w[d], piece by piece
        xw = sp.tile([P, L], F32)
        for i in range(NP):
            x3 = xts[i][:].rearrange("p (l d) -> p l d", d=D)
            eng = nc.gpsimd if i in MUL_ON_GPSIMD else nc.vector
            eng.tensor_tensor(x3, x3, w3, mult)
            nc.vector.tensor_reduce(
                xw[:, i * LP:(i + 1) * LP], x3, axis=AX, op=add
            )

        # masked exp, chunk totals, cross-chunk exclusive suffix via matmul
        em = sp.tile([P, L], F32)
        nc.scalar.activation(em[:], xw[:], ACT.Exp)
        # dummy Ln pulls the Ln table load off the serial tail; reading em
        # pins it between the exp above and the real Ln below so the
        # scheduler can't hoist it (which would evict the Exp table)
        lnwarm = sp.tile([P, 1], F32)
        nc.scalar.activation(lnwarm[:], em[:, 0:1], ACT.Ln)
        nc.vector.tensor_mul(em[:], em[:], mf[:])
        tot = sp.tile([P, 1], F32)
        nc.vector.tensor_reduce(tot[:], em[:], axis=AX, op=add)
        aps = pp.tile([P, 1], F32, tag="mm")
        nc.tensor.matmul(aps[:], u_sb[:], tot[:], start=True, stop=True)
        a_sb = sp.tile([P, 1], F32)
        # + EPS seeds every suffix sum, keeping ln() finite on
        # fully-masked tails
        nc.vector.tensor_scalar_add(a_sb[:], aps[:], EPS)

        # within-chunk suffix sums, seeded with the later-chunk total
        ss = sp.tile([P, L], F32)
        nc.vector.tensor_tensor_scan(
            ss[:][:, ::-1], em[:][:, ::-1], em[:][:, ::-1],
            initial=a_sb[:], op0=add, op1=bypass,
        )
        lt = sp.tile([P, L], F32)
        nc.scalar.activation(lt[:], ss[:], ACT.Ln)

        # loss terms: sum over valid groups of (ln(suffix) - xw), and count
        wm = sp.tile([P, L], F32)
        nc.gpsimd.tensor_mul(wm[:], mf[:], cm_sb[:])
        diff = sp.tile([P, L], F32)
        nc.vector.tensor_sub(diff[:], lt[:], xw[:])
        res = sp.tile([P, 2], F32)
        nc.vector.scalar_tensor_tensor(
            out=diff[:], in0=diff[:], scalar=1.0, in1=wm[:],
            op0=bypass, op1=mult, accum_out=res[:, 0:1],
        )
        nc.vector.tensor_reduce(res[:, 1:2], mf[:], axis=AX, op=add)
        nc.sync.dma_start(out, res[:])

    nc.compile()
    return nc


def _host_consts():
    w_idx = np.arange(P)
    um = (
        (w_idx[:, None] // CH == w_idx[None, :] // CH)
        & (w_idx[:, None] % CH > w_idx[None, :] % CH)
    ).astype(np.float32)
    cm = np.ones((P, L), np.float32)
    cm[w_idx % CH == 0, 0] = 0.0
    return um, cm


def kernel(**inputs) -> np.ndarray:
    enc = np.ascontiguousarray(np.asarray(inputs["encoder_output"], np.float32))
    mask = np.ascontiguousarray(np.asarray(inputs["mask"], np.int32))
    w_fc = np.asarray(inputs["w_fc"], np.float32)

    if "nc" not in _cache:
        _cache["nc"] = _build_nc()
    nc = _cache["nc"]

    wt = np.ascontiguousarray(np.broadcast_to(w_fc[HID:], (P, D)), np.float32)
    um, cm = _host_consts()
    in_maps = [
        {
            "x": enc[c * BS:(c + 1) * BS],
            "mk": mask[c * BS:(c + 1) * BS],
            "wt": wt,
            "um": um,
            "cm": cm,
        }
        for c in range(NCORES)
    ]
    res = bass_utils.run_bass_kernel_spmd(
        nc, in_maps, core_ids=list(range(NCORES))
    )
    o = np.stack([r["out"] for r in res.results]).astype(np.float64)
    num = o[:, :, 0].sum()
    den = o[:, :, 1].sum() - B
    return np.asarray(num / den, dtype=np.float32)



# revision 6
# speedup vs baseline: 1.2162x; 1.2162x over previous
"""Trainium2 Bass kernel for nn_DLI_loss_full.

Key algebraic fact: logits[b,j,k] = hw[b,j] + xw[b,k] and the loss is
sum(lse - tgt) over valid groups, so the hw[b,j] term (the whole LSTM
path) cancels exactly:

    per_group[b,j] = log(sum_{k=j+1}^{len_b-1} exp(xw[b,k])) - xw[b,j+1]
    loss = sum(per_group) / sum_b(len_b - 1)

with xw = encoder_output @ w_fc[HID:].  The kernel only streams
encoder_output once (memory-bound).

v2 changes vs the 54-57us baseline (trace-driven):
  * stream DMAs cast f32->bf16 in the SDMA datapath (SWDGE/gpsimd path;
    HBM read bytes unchanged, SBUF writes halved) so the multiply runs
    on DVE in bf16 2x perf mode (~1.0us/piece vs 1.75 fp32, and vs
    3.4us/piece on gpsimd where the old kernel put 5 of 8 pieces).
  * reduction over d=256 done as 2 bf16 tree-add steps (2x mode) plus a
    64-wide tensor_reduce: ~1.37us/piece vs 1.66 plain; two pieces'
    reductions go to the otherwise idle gpsimd engine.
  * mask folded into xw additively ((mf-1)*30) before a single
    exp-with-accumulate on the scalar engine: the exp's accum_out IS the
    chunk total, killing the separate mask-mul + reduce, and no EPS foot
    guard is needed (masked exps are e^-30, not 0, so ln stays finite).
  * cross-chunk suffix combine matmul runs in bf16 (one PE pass, not
    the fp32r multi-pass).
  * the Ln activation-table load is triggered by a dummy Ln on a memset
    tile right after the real exp, so the ~1.3us table load overlaps the
    matmul/scan chain instead of sitting on the critical path (the old
    kernel's warm read em in-place and stalled the DVE on a WAR hazard).
  * per-engine queues ordered so nothing program-order-blocks: consts on
    the sync HWDGE queue, stream issues on gpsimd, DVE does
    mult->tree->reduce per piece in arrival order.
"""

from contextlib import ExitStack

import numpy as np

import concourse.bacc as bacc
import concourse.mybir as mybir
import concourse.tile as tile
from concourse import bass_utils

B, T, D, HID = 128, 384, 256, 256
NCORES = 8
BS = B // NCORES            # 16 batches per core
CH = 8                      # chunks per sequence
L = T // CH                 # 48 timesteps per chunk
P = BS * CH                 # 128 partitions
NP = 8                      # DMA/compute pieces along the free axis
LP = L // NP                # 6 timesteps per piece
F32 = mybir.dt.float32
BF16 = mybir.dt.bfloat16
I32 = mybir.dt.int32
NEGM = 30.0                 # additive mask depth: exp(xw-30) ~ 1e-13

# pieces whose elementwise multiply runs on gpsimd (frees DVE cycles;
# gpsimd's tensor_reduce is partition-axis only, so the reduce stays
# on DVE for every piece)
MUL_ON_GPSIMD = (1, 4)

_cache = {}


def _build_nc():
    nc = bacc.Bacc(
        "TRN2", target_bir_lowering=False, debug=False, num_devices=NCORES
    )
    x = nc.dram_tensor("x", [BS, T, D], F32, kind="ExternalInput").ap()
    mk = nc.dram_tensor("mk", [BS, T], I32, kind="ExternalInput").ap()
    wt = nc.dram_tensor("wt", [P, D], F32, kind="ExternalInput").ap()
    um = nc.dram_tensor("um", [P, P], F32, kind="ExternalInput").ap()
    cm = nc.dram_tensor("cm", [P, L], F32, kind="ExternalInput").ap()
    out = nc.dram_tensor("out", [P, 2], F32, kind="ExternalOutput").ap()

    add = mybir.AluOpType.add
    mult = mybir.AluOpType.mult
    subtract = mybir.AluOpType.subtract
    bypass = mybir.AluOpType.bypass
    AX = mybir.AxisListType.X
    ACT = mybir.ActivationFunctionType

    with tile.TileContext(nc) as tc, ExitStack() as ctx:
        sp = ctx.enter_context(tc.tile_pool(name="small", bufs=1))
        xp = ctx.enter_context(tc.tile_pool(name="xp", bufs=NP))
        pp = ctx.enter_context(tc.tile_pool(name="psum", bufs=1, space="PSUM"))

        # constants ride the sync HWDGE queue (idle engine, low first-byte
        # latency); w first since it gates the whole DVE pipeline
        w_sb = sp.tile([P, D], F32)
        nc.sync.dma_start(w_sb[:], wt)
        mi = sp.tile([P, L], I32)
        nc.sync.dma_start(mi[:], mk.rearrange("b (c l) -> (b c) l", c=CH))
        cm_sb = sp.tile([P, L], F32)
        nc.sync.dma_start(cm_sb[:], cm)
        u_sb = sp.tile([P, P], F32)
        nc.sync.dma_start(u_sb[:], um)

        # the big stream: 8 pieces, f32 in DRAM -> bf16 in SBUF (SWDGE
        # cast path; per-partition DRAM source is 6KB contiguous)
        x_p = x.rearrange("b (c n l) d -> (b c) n (l d)", c=CH, n=NP)
        xts = []
        for i in range(NP):
            xt = xp.tile([P, LP * D], BF16, tag="x")
            nc.gpsimd.dma_start(xt[:], x_p[:, i, :])
            xts.append(xt)

        # activation-table warm tile: no data dependency, so the Exp
        # table load runs during the DMA phase
        warm0 = sp.tile([P, 1], F32)
        nc.vector.memset(warm0[:], 1.0)
        warmo = sp.tile([P, 4], F32)
        nc.scalar.activation(warmo[:, 0:1], warm0[:], ACT.Exp)

        # bf16 copies of the small operands
        w_bf = sp.tile([P, D], BF16)
        nc.vector.tensor_copy(w_bf[:], w_sb[:])
        wrep = sp.tile([P, LP * D], BF16)
        nc.vector.tensor_copy(wrep[:, 0:D], w_bf[:])
        nc.vector.tensor_copy(wrep[:, D:2 * D], wrep[:, 0:D])
        nc.vector.tensor_copy(wrep[:, 2 * D:4 * D], wrep[:, 0:2 * D])
        nc.vector.tensor_copy(wrep[:, 4 * D:6 * D], wrep[:, 2 * D:4 * D])
        w3 = wrep[:].rearrange("p (l d) -> p l d", d=D)
        u_bf = sp.tile([P, P], BF16)

        # mask-derived small tensors on gpsimd (after its DMA issues)
        mf = sp.tile([P, L], F32)
        nc.gpsimd.tensor_copy(mf[:], mi[:])
        wm = sp.tile([P, L], F32)
        nc.gpsimd.tensor_mul(wm[:], mf[:], cm_sb[:])
        amask = sp.tile([P, L], F32)
        nc.gpsimd.tensor_scalar(amask[:], mf[:], NEGM, NEGM, mult, subtract)

        # xw[p, t] = sum_d x[p, t, d] * w[d]: bf16 multiply (2x mode),
        # two bf16 tree-add halvings (2x), then a 64-wide reduce.
        # gpsimd multiplies pieces 1 and 4; their (DVE) tree/reduce is
        # deferred a couple of pieces so a slow gpsimd multiply never
        # program-order-blocks the DVE queue.
        xw = sp.tile([P, L], F32)
        res = sp.tile([P, 2], F32)
        h1 = sp.tile([P, LP * 128], BF16)
        h13 = h1[:].rearrange("p (l d) -> p l d", d=128)
        h2 = sp.tile([P, LP * 64], BF16)
        h23 = h2[:].rearrange("p (l d) -> p l d", d=64)
        x3s = [xts[i][:].rearrange("p (l d) -> p l d", d=D) for i in range(NP)]

        def emit_mult(i):
            eng = nc.gpsimd if i in MUL_ON_GPSIMD else nc.vector
            eng.tensor_tensor(x3s[i], x3s[i], w3, mult)

        def emit_tree_red(i):
            x3 = x3s[i]
            nc.vector.tensor_tensor(h13, x3[:, :, 0:128], x3[:, :, 128:256], add)
            nc.vector.tensor_tensor(h23, h13[:, :, 0:64], h13[:, :, 64:128], add)
            nc.vector.tensor_reduce(xw[:, i * LP:(i + 1) * LP], h23, axis=AX, op=add)

        emit_mult(0)
        emit_tree_red(0)
        emit_mult(1)            # gpsimd
        emit_mult(2)
        emit_tree_red(2)
        emit_tree_red(1)
        # group count + bf16 um for the later matmul: DVE has slack here
        nc.vector.tensor_reduce(res[:, 1:2], mf[:], axis=AX, op=add)
        nc.vector.tensor_copy(u_bf[:], u_sb[:])
        emit_mult(3)
        emit_tree_red(3)
        emit_mult(4)            # gpsimd
        emit_mult(5)
        emit_tree_red(5)
        emit_mult(6)
        emit_tree_red(6)
        emit_tree_red(4)
        emit_mult(7)
        emit_tree_red(7)

        # fold the mask in additively: valid cols unchanged, masked cols
        # pushed to ~-30 so exp gives ~1e-13 (keeps every suffix sum > 0)
        nc.vector.tensor_tensor(xw[:], xw[:], amask[:], add)

        # masked exponentials; the accumulate IS the chunk total
        em = sp.tile([P, L], F32)
        tot = sp.tile([P, 1], F32)
        nc.scalar.activation(em[:], xw[:], ACT.Exp, accum_out=tot[:])
        # dummy Ln pulls the Ln table load off the critical path; it reads
        # the memset tile so the DVE never waits on it
        nc.scalar.activation(warmo[:, 1:2], warm0[:], ACT.Ln)

        # cross-chunk exclusive suffix of totals via one bf16 matmul
        tot_bf = sp.tile([P, 1], BF16)
        nc.vector.tensor_copy(tot_bf[:], tot[:])
        aps = pp.tile([P, 1], F32, tag="mm")
        nc.tensor.matmul(aps[:], u_bf[:], tot_bf[:], start=True, stop=True)
        a_sb = sp.tile([P, 1], F32)
        nc.vector.tensor_copy(a_sb[:], aps[:])

        # within-chunk suffix sums, seeded with the later-chunk total
        ss = sp.tile([P, L], F32)
        nc.vector.tensor_tensor_scan(
            ss[:][:, ::-1], em[:][:, ::-1], em[:][:, ::-1],
            initial=a_sb[:], op0=add, op1=bypass,
        )
        lt = sp.tile([P, L], F32)
        nc.scalar.activation(lt[:], ss[:], ACT.Ln)

        # loss terms: sum over valid groups of (ln(suffix) - xw); the
        # amask offset only lives where wm == 0, so it never contributes
        diff = sp.tile([P, L], F32)
        nc.vector.tensor_sub(diff[:], lt[:], xw[:])
        nc.vector.scalar_tensor_tensor(
            out=diff[:], in0=diff[:], scalar=1.0, in1=wm[:],
            op0=bypass, op1=mult, accum_out=res[:, 0:1],
        )
        nc.sync.dma_start(out, res[:])

    nc.compile()
    return nc


def _host_consts():
    w_idx = np.arange(P)
    um = (
        (w_idx[:, None] // CH == w_idx[None, :] // CH)
        & (w_idx[:, None] % CH > w_idx[None, :] % CH)
    ).astype(np.float32)
    cm = np.ones((P, L), np.float32)
    cm[w_idx % CH == 0, 0] = 0.0
    return um, cm


def kernel(**inputs) -> np.ndarray:
    enc = np.ascontiguousarray(np.asarray(inputs["encoder_output"], np.float32))
    mask = np.ascontiguousarray(np.asarray(inputs["mask"], np.int32))
    w_fc = np.asarray(inputs["w_fc"], np.float32)

    if "nc" not in _cache:
        _cache["nc"] = _build_nc()
    nc = _cache["nc"]

    wt = np.ascontiguousarray(np.broadcast_to(w_fc[HID:], (P, D)), np.float32)
    um, cm = _host_consts()
    in_maps = [
        {
            "x": enc[c * BS:(c + 1) * BS],
            "mk": mask[c * BS:(c + 1) * BS],
            "wt": wt,
            "um": um,
            "cm": cm,
        }
        for c in range(NCORES)
    ]
    res = bass_utils.run_bass_kernel_spmd(
        nc, in_maps, core_ids=list(range(NCORES))
    )
    o = np.stack([r["out"] for r in res.results]).astype(np.float64)
    num = o[:, :, 0].sum()
    den = o[:, :, 1].sum() - B
    return np.asarray(num / den, dtype=np.float32)
